# revision 11
# baseline (speedup 1.0000x reference)
"""ConvCrossAttention Trainium2 kernel — self-contained.

Problem (B=4, C_in=C_out=256, H=W=64, N=4096):
  q = conv1x1(x1, Wq, bq); k = conv1x1(x2, Wk, bk); v = conv1x1(x2, Wv, bv)
  out = softmax(q^T k / sqrt(C)) @ v^T, back in conv layout [B, C, H, W].

Sharding: data-parallel over (batch, query-half) -> 8 NeuronCores.
Core c handles batch c//2, query rows (c%2)*2048 : (c%2+1)*2048, with the
full 4096-key context for that batch. No collectives.

Algebraic restructure (host-side weight folding only):
  softmax is invariant to per-query additive constants, so
    scores = (Wq x1 + bq)^T (Wk x2 + bk) / 16
           ~ x1^T M' x2 + alpha_m,   M' = Wq^T Wk / 16,
    alpha = u'^T x2,                 u' = Wk^T bq / 16
  (all bk terms and the bq.bk constant drop out per-query).
  So on device only the SMALL side is projected: q' = M'^T x1 (2048 cols
  vs 4096), the key side uses raw x2 as the S-matmul stationary operand,
  and alpha rides along as extra output columns of the V projection
  (u' appended to Wv^T) -> it lands per-partition-per-key-tile, exactly
  the layout the ACT exp wants for its per-partition bias.

Per-core program (everything SBUF-resident):
  q'[c, nq]   = M'^T @ X1                   (PE, f32r)
  Vx[nk, c+2] = X2[:, nk].T @ [Wv^T | u']   (PE; col 256 = alpha)
  per 512-wide nq chunk, software-pipelined over 32 nk tiles:
    S^T[nk, nq] = X2[:, t].T @ q'[:, chunk] (PE f32r -> PSUM, 2 K-steps)
    P = exp(S^T + alpha_t)                  (ACT, bf16 out, bias AP;
                                             no max-sub: |scores| < ~7)
    acc[c, nq] += Vx[t, :256].T @ P         (PE bf16, PSUM accumulate)
    P-sums += P                             (Pool/DVE alternating, bf16
                                             so DVE runs in 2x mode)
  tail per chunk (emitted one chunk late so the PE queue never stalls on
  the DVE chain): den = ones.T @ P-sum (PE); recip_approx_fast(1/den);
  bcast = ones_row.T @ (1/den) (PE); out = acc * bcast + bv -> DMA.

Numerics: the score path (x1, x2, q', M') is float32r (fp32 bit layout,
PE truncates to ~tf32) because score error passes through exp() into the
attention weights; the value path (P, Vx) is bf16, whose ~0.1% RMS
quantization averages out over ~10^3 effective keys. End-to-end rel err
~1e-3 vs the 2e-2 gate. Weights ship as one packed [256, 515] tensor
(M' | Wv^T,u',pad | bv), startup DMAs are issued priority-first with
small leading pieces so the first matmul lands ~7us after kernel start.
"""

import sys

if "/opt/trn_rl_repo" not in sys.path:
    sys.path.insert(0, "/opt/trn_rl_repo")

from contextlib import ExitStack

import numpy as np

import concourse.bass as bass  # noqa: F401  (engine types referenced via nc)
import concourse.mybir as mybir
import concourse.tile as tile
from concourse import bacc
from concourse.bass_utils import run_bass_kernel_spmd

F32 = mybir.dt.float32
F32R = mybir.dt.float32r
BF16 = mybir.dt.bfloat16

B, C, H, W = 4, 256, 64, 64
N = H * W  # 4096
NQ = 2048  # queries per core (half a batch)
NK = 4096  # full key context
CHUNK = 512
NQ_CHUNKS = NQ // CHUNK
NK_TILES = NK // 128
PIPE = 2  # PV matmuls trail S matmuls by this many nk tiles
WCOLS = C + (C + 2) + 1  # M' | WvT,u',pad | bv = 515 (V dst width even for fp32r)
XK_PIECES = [256, 256, 512, 512, 512, 512, 512, 512, 512]  # small first pieces
XQ_PIECES = [256, 256, 512, 512, 512]


def build_nc():
    MM = F32R
    nc = bacc.Bacc(None, debug=False)

    xq = nc.dram_tensor("xq", [C, NQ], MM, kind="ExternalInput")
    xk = nc.dram_tensor("xk", [C, NK], MM, kind="ExternalInput")
    wp = nc.dram_tensor("wp", [C, WCOLS], MM, kind="ExternalInput")
    out = nc.dram_tensor("out", [C, NQ], F32, kind="ExternalOutput")

    with tile.TileContext(nc) as tc, ExitStack() as ctx:
        big = ctx.enter_context(tc.tile_pool(name="big", bufs=1))
        small = ctx.enter_context(tc.tile_pool(name="small", bufs=1))
        ppool = ctx.enter_context(tc.tile_pool(name="p", bufs=6))
        opool = ctx.enter_context(tc.tile_pool(name="o", bufs=4))
        dpool = ctx.enter_context(tc.tile_pool(name="d", bufs=2))
        spsum = ctx.enter_context(tc.tile_pool(name="spsum", bufs=3, space="PSUM"))
        apsum = ctx.enter_context(tc.tile_pool(name="apsum", bufs=4, space="PSUM"))
        dpsum = ctx.enter_context(tc.tile_pool(name="dpsum", bufs=1, space="PSUM"))

        # --- weights (one packed tensor, 1 DMA) ---
        wp_sb = small.tile([128, 2, WCOLS], MM, tag="wp")
        nc.sync.dma_start(
            out=wp_sb[:, :, 0:C], in_=wp[:, 0:C].rearrange("(h p) c -> p h c", h=2)
        )

        mslice = lambda h, ct: wp_sb[:, h, ct * 128 : (ct + 1) * 128]  # noqa: E731
        wv_sl = lambda h: wp_sb[:, h, C : C + C + 2]  # noqa: E731
        bv_sl = lambda ct: wp_sb[:, ct, WCOLS - 1 : WCOLS]

        ones_col_f32 = small.tile([128, 1], F32, tag="ones_col_f32")
        nc.vector.memset(ones_col_f32[:], 1.0)
        ones_col = small.tile([128, 1], MM, tag="ones_col")
        nc.vector.tensor_copy(ones_col[:], ones_col_f32[:])
        ones_row_f32 = small.tile([1, 128], F32, tag="ones_row_f32")
        nc.vector.memset(ones_row_f32[:], 1.0)
        ones_row = small.tile([1, 128], MM, tag="ones_row")
        nc.vector.tensor_copy(ones_row[:], ones_row_f32[:])
        ones_w = small.tile([128, 2], MM, tag="ones_w")
        nc.vector.tensor_copy(ones_w[:, 0:1], ones_col_f32[:])
        nc.vector.tensor_copy(ones_w[:, 1:2], ones_col_f32[:])

        # --- big SBUF residents ---
        x2_sb = big.tile([128, 2, NK], MM, tag="x2")
        x1_sb = big.tile([128, 2, NQ], MM, tag="x1")
        q_sb = big.tile([128, 2, NQ], MM, tag="q")
        va_sb = big.tile([128, NK_TILES, C + 2], MM, tag="va")

        def dma_piece(dst, src, pieces, j):
            c0 = sum(pieces[:j])
            cs = slice(c0, c0 + pieces[j])
            nc.sync.dma_start(
                out=dst[:, :, cs],
                in_=src[:, cs].rearrange("(h p) c -> p h c", h=2),
            )

        def vproj(tiles):
            for t in tiles:
                ts = slice(t * 128, (t + 1) * 128)
                vp = spsum.tile([128, C + 2], F32, tag="s", name="vp")
                nc.tensor.matmul(
                    vp[:], x2_sb[:, 0, ts], wv_sl(0), start=True, stop=False
                )
                nc.tensor.matmul(
                    vp[:], x2_sb[:, 1, ts], wv_sl(1), start=False, stop=True
                )
                nc.scalar.copy(va_sb[:, t, :], vp[:])

        def qproj(j):
            c0 = sum(XQ_PIECES[:j])
            cs = slice(c0, c0 + XQ_PIECES[j])
            for ct in range(2):
                qp = spsum.tile([128, CHUNK], F32, tag="s", name="qp")
                qps = qp[:, 0 : XQ_PIECES[j]]
                nc.tensor.matmul(
                    qps, mslice(0, ct), x1_sb[:, 0, cs], start=True, stop=False
                )
                nc.tensor.matmul(
                    qps, mslice(1, ct), x1_sb[:, 1, cs], start=False, stop=True
                )
                nc.vector.tensor_copy(q_sb[:, ct, cs], qps)

        # priority-first DMA issue: weights, first xq piece, then xk/xq
        # pieces interleaved with the projection matmuls that consume them.
        dma_piece(x1_sb, xq, XQ_PIECES, 0)
        # PE warmup: N=512 matmuls (reading a not-yet-written SBUF region,
        # contents irrelevant) fill the DMA-wait window so the HAM clock gate
        # is already released (2.4 GHz) when the first real matmul runs.
        warm = dpsum.tile([1, CHUNK], F32, tag="d", name="warm")
        warm_rhs = q_sb[:, 0, NQ - CHUNK : NQ]
        for _ in range(10):
            nc.tensor.matmul(warm[:], ones_col[:], warm_rhs, start=True, stop=True)
        nc.sync.dma_start(
            out=wp_sb[:, :, C:WCOLS],
            in_=wp[:, C:WCOLS].rearrange("(h p) c -> p h c", h=2),
        )
        dma_piece(x2_sb, xk, XK_PIECES, 0)
        dma_piece(x1_sb, xq, XQ_PIECES, 1)
        qproj(0)
        qproj(1)
        for _ in range(6):
            nc.tensor.matmul(warm[:], ones_col[:], warm_rhs, start=True, stop=True)
        xk_tile = 0
        for j in range(1, len(XK_PIECES)):
            dma_piece(x2_sb, xk, XK_PIECES, j)
            ntile = XK_PIECES[j - 1] // 128
            vproj(range(xk_tile, xk_tile + ntile))
            xk_tile += ntile
        vproj(range(xk_tile, NK_TILES))
        for j in range(2, len(XQ_PIECES)):
            dma_piece(x1_sb, xq, XQ_PIECES, j)
            qproj(j)

        # --- attention; each chunk's tail is emitted one chunk late so
        # the PE queue never stalls on the DVE reciprocal chain ---
        tail_a = tail_b = None
        for c0 in range(NQ_CHUNKS):
            cs = slice(c0 * CHUNK, (c0 + 1) * CHUNK)
            acc0 = apsum.tile([128, CHUNK], F32, tag="acc", name="acc0")
            acc1 = apsum.tile([128, CHUNK], F32, tag="acc", name="acc1")
            # P-sum split across Pool (t%3==0) and DVE (rest; bf16 2x mode
            # makes DVE ~2x faster) so neither serial chain gates the PE.
            psum_p = dpool.tile([128, CHUNK], MM, tag="psum_p", name="psum_p")
            psum_d = dpool.tile([128, CHUNK], MM, tag="psum_d", name="psum_d")
            p_tiles = {}

            def emit_pv(t, acc0=acc0, acc1=acc1, psum_p=psum_p, psum_d=psum_d, p_tiles=p_tiles):
                first, last = t == 0, t == NK_TILES - 1
                p = p_tiles.pop(t)
                nc.tensor.matmul(
                    acc0[:], va_sb[:, t, 0:128], p[:], start=first, stop=last
                )
                nc.tensor.matmul(
                    acc1[:], va_sb[:, t, 128:256], p[:], start=first, stop=last
                )
                eng, acc_ps = (
                    (nc.gpsimd, psum_p) if t % 3 == 0 and t > 0 else (nc.vector, psum_d)
                )
                if t < 2:
                    # init both accumulators on DVE (GpSimd copies are slow)
                    acc_ps = psum_p if t == 0 else psum_d
                    nc.vector.tensor_copy(acc_ps[:], p[:].bitcast(F32))
                else:
                    eng.tensor_add(
                        acc_ps[:], acc_ps[:].bitcast(F32), p[:].bitcast(F32)
                    )

            for t in range(NK_TILES):
                ts = slice(t * 128, (t + 1) * 128)
                sp = spsum.tile([128, CHUNK], F32, tag="s", name="sp")
                nc.tensor.matmul(
                    sp[:], x2_sb[:, 0, ts], q_sb[:, 0, cs], start=True, stop=False
                )
                nc.tensor.matmul(
                    sp[:], x2_sb[:, 1, ts], q_sb[:, 1, cs], start=False, stop=True
                )
                p = ppool.tile([128, CHUNK], MM, tag="p", name="p")
                nc.scalar.activation(
                    p[:],
                    sp[:],
                    mybir.ActivationFunctionType.Exp,
                    bias=va_sb[:, t, C : C + 1].bitcast(F32),
                )
                p_tiles[t] = p
                if t >= PIPE:
                    emit_pv(t - PIPE)
                if t == 12 and tail_a is not None:
                    tail_a()
                if t == 22 and tail_b is not None:
                    tail_b()

            for t in range(NK_TILES - PIPE, NK_TILES):
                emit_pv(t)

            def tail_a(acc0=acc0, acc1=acc1, psum_p=psum_p, psum_d=psum_d, cs=cs):
                # denominator: one partition-reduction matmul per chunk
                den = dpsum.tile([1, CHUNK], F32, tag="d", name="den")
                nc.tensor.matmul(
                    den[:], ones_col[:], psum_p[:], start=True, stop=False
                )
                nc.tensor.matmul(
                    den[:], ones_col[:], psum_d[:], start=False, stop=True
                )
                recip_f32 = dpool.tile([1, CHUNK], F32, tag="recip_f32", name="recip_f32")
                nc.vector.reciprocal_approx_fast(recip_f32[:], den[:])
                recip_sb = dpool.tile([1, CHUNK], MM, tag="recip_sb", name="recip_sb")
                nc.vector.tensor_copy(recip_sb[:], recip_f32[:])
                tail_a.recip_sb = recip_sb[:]

            def tail_b(acc0=acc0, acc1=acc1, cs=cs, tail_a=tail_a):
                recip_sb = tail_a.recip_sb
                bcast = dpsum.tile([128, CHUNK], F32, tag="d", name="bcast")
                nc.tensor.matmul(
                    bcast[:], ones_row[:], recip_sb[:], start=True, stop=True
                )
                bcast_sb = opool.tile([128, CHUNK], F32, tag="ob", name="bcast_sb")
                nc.vector.tensor_copy(bcast_sb[:], bcast[:])
                for ct, acc in ((0, acc0), (1, acc1)):
                    tmp = opool.tile([128, CHUNK], F32, tag="ob", name="tmp")
                    nc.vector.tensor_mul(tmp[:], acc[:], bcast_sb[:])
                    o = opool.tile([128, CHUNK], F32, tag="o", name="o")
                    nc.vector.tensor_scalar_add(o[:], tmp[:], bv_sl(ct).bitcast(F32))
                    nc.sync.dma_start(
                        out=out[ct * 128 : (ct + 1) * 128, cs], in_=o[:]
                    )

        # final chunk's tail
        tail_a()
        tail_b()

    nc.compile()
    return nc


def core_inputs(inputs, core):
    """Slice full-problem inputs for one core (numpy)."""
    b, h = core // 2, core % 2
    x1r = np.asarray(inputs["x1"], dtype=np.float32).reshape(B, C, N)
    x2r = np.asarray(inputs["x2"], dtype=np.float32).reshape(B, C, N)
    wq = np.asarray(inputs["Wq"], dtype=np.float64)
    wk = np.asarray(inputs["Wk"], dtype=np.float64)
    wv = np.asarray(inputs["Wv"], dtype=np.float64)
    bq = np.asarray(inputs["bq"], dtype=np.float64)
    bv = np.asarray(inputs["bv"], dtype=np.float64)
    scale = 1.0 / np.sqrt(C)
    mprime = wq.T @ wk * scale  # [c1, c2]
    uprime = wk.T @ bq * scale  # [c2]
    wpack = np.concatenate(
        [mprime, wv.T, uprime[:, None], uprime[:, None], bv[:, None]], axis=1
    ).astype(np.float32)  # [256, 515]
    return {
        "xq": np.ascontiguousarray(x1r[b][:, h * NQ : (h + 1) * NQ]),
        "xk": np.ascontiguousarray(x2r[b]),
        "wp": np.ascontiguousarray(wpack),
    }


_NC_CACHE = {}


def get_nc():
    if "nc" not in _NC_CACHE:
        _NC_CACHE["nc"] = build_nc()
    return _NC_CACHE["nc"]


def assemble(results) -> np.ndarray:
    """Gather per-core outputs into the full [4,256,64,64] f32 tensor."""
    full = np.zeros((B, C, N), np.float32)
    for core in range(8):
        b, h = core // 2, core % 2
        full[b][:, h * NQ : (h + 1) * NQ] = results[core]["out"]
    return full.reshape(B, C, H, W)


def kernel(**inputs) -> np.ndarray:
    """Full-problem entry point: full inputs in, full [4,256,64,64] f32 out."""
    nc = get_nc()
    in_maps = [core_inputs(inputs, core) for core in range(8)]
    res = run_bass_kernel_spmd(nc, in_maps, list(range(8)))
    return assemble(res.results)
